# revision 1
# baseline (speedup 1.0000x reference)
"""SimCLR contrastive loss on 8 TRN2 NeuronCores.

Strategy (per spec sharding_hint): shard the N=8192 anchors row-wise across
8 cores; replicate the normalized pred/positive matrices. Normalization and
transposition are cheap O(N*D) host work; the O(N^2) similarity + exp +
row-reduction runs on device and never materializes the NxN matrices.

Host side (in kernel()):
  - L2-normalize rows of pred/positive (torch-style eps clamp).
  - s[i] = zp_i . zq_i  (positive-pair logit, exact diag of the pq matrix).
  - Build zpT/zqT = normalized matrices transposed to [D=128, N=8192], with
    columns rolled per core so each core's own 1024 anchor columns come
    first — the SPMD program is identical on all cores.

Device side (per core, identical program):
  - DMA zpT/zqT into SBUF as float32r (full-rate fp32 TensorEngine mode).
  - For each own 128-row chunk m: S-block = zpT[:, m-block].T @ Z*T against
    all 8192 columns (16 matmuls of [128,512] per matrix into PSUM).
  - ScalarE: exp(2*S) with accum_out => per-row partial sums, 2048 columns
    per ACTIVATE (4 PSUM banks). Only row sums leave the engine.
  - DMA out [128, 64] partial sums (8 m-chunks x 8 groups).

Host finish: neg_i = sum(partials_i) - e^2 (removes the pp diagonal,
exp(2*cos(x,x)) = e^2);  loss_i = log(neg_i) - 2*s_i;  mean over rows.
"""

import numpy as np

N = 8192
D = 128
P = 128
NCORES = 8
M_LOCAL = N // NCORES          # 1024 own rows per core
T_OWN = M_LOCAL // P           # 8 own row chunks
NB = 512                       # matmul moving free dim (one PSUM bank of f32)
GRP = 2048                     # columns per ACT exp instruction (4 banks)
N_GRP = N // GRP               # 4 groups per matrix per row-chunk
OUT_COLS = T_OWN * 2 * N_GRP   # 64 accum columns

EPS = 1e-8
TEMP = 0.5

_CACHE = {}


def _build_nc():
    import concourse.mybir as mybir
    from concourse import bacc
    from concourse.tile import TileContext
    from contextlib import ExitStack

    f32 = mybir.dt.float32
    f32r = mybir.dt.float32r
    AF = mybir.ActivationFunctionType

    nc = bacc.Bacc()
    zpt_d = nc.dram_tensor("zpt", [P, N], f32r, kind="ExternalInput")
    zqt_d = nc.dram_tensor("zqt", [P, N], f32r, kind="ExternalInput")
    out_d = nc.dram_tensor("out", [P, OUT_COLS], f32, kind="ExternalOutput")

    with TileContext(nc) as tc:
        with ExitStack() as ctx:
            sbuf = ctx.enter_context(tc.tile_pool(name="sbuf", bufs=1))
            zpT = sbuf.tile([P, N], f32r)
            zqT = sbuf.tile([P, N], f32r)
            outt = sbuf.tile([P, OUT_COLS], f32)

            # chunked loads so the first matmuls start after ~1 MiB lands
            for g in range(N_GRP):
                cs = slice(g * GRP, (g + 1) * GRP)
                nc.sync.dma_start(out=zpT[:, cs], in_=zpt_d[:, cs])
            for g in range(N_GRP):
                cs = slice(g * GRP, (g + 1) * GRP)
                nc.sync.dma_start(out=zqT[:, cs], in_=zqt_d[:, cs])

            ps_pool = ctx.enter_context(
                tc.tile_pool(name="ps_pool", bufs=2, space="PSUM"))
            scr_pool = ctx.enter_context(tc.tile_pool(name="scr_pool", bufs=2))
            # all pp row-chunks first (needs only zpt), then all pq — the
            # zqt DMA has the whole pp phase (~60us of ACT work) to land
            for mi, zT in enumerate((zpT, zqT)):
                for m in range(T_OWN):
                    lhsT = zpT[:, m * P:(m + 1) * P]
                    for g in range(N_GRP):
                        pt = ps_pool.tile([P, GRP], f32, tag="ps")
                        for s in range(GRP // NB):
                            col = g * GRP + s * NB
                            nc.tensor.matmul(
                                pt[:, s * NB:(s + 1) * NB],
                                lhsT=lhsT,
                                rhs=zT[:, col:col + NB],
                                start=True, stop=True,
                            )
                        scr = scr_pool.tile([P, GRP], f32, tag="scr")
                        acc_col = m * 8 + mi * N_GRP + g
                        nc.scalar.activation(
                            scr[:, :], pt[:, :], AF.Exp, scale=2.0,
                            accum_out=outt[:, acc_col:acc_col + 1],
                        )

            nc.sync.dma_start(out=out_d[:, :], in_=outt[:, :])

    nc.finalize()
    return nc


def _get_nc():
    if "nc" not in _CACHE:
        _CACHE["nc"] = _build_nc()
    return _CACHE["nc"]


def _host_prep(pred, positive):
    """Normalize rows, compute positive-pair logits, build transposed
    per-core (column-rolled) input matrices."""
    def nrm(x):
        n = np.sqrt(np.sum(x * x, axis=1, keepdims=True))
        return x / np.maximum(n, np.float32(EPS))

    zp = nrm(pred)
    zq = nrm(positive)
    s = np.sum(zp.astype(np.float64) * zq.astype(np.float64), axis=1)
    zpT = np.ascontiguousarray(zp.T)   # [D, N]
    zqT = np.ascontiguousarray(zq.T)
    return zpT, zqT, s


LAST_RESULTS = None


def kernel(pred: np.ndarray, positive: np.ndarray) -> np.ndarray:
    global LAST_RESULTS
    import sys
    if "/opt/trn_rl_repo" not in sys.path:
        sys.path.insert(0, "/opt/trn_rl_repo")
    from concourse.bass_utils import run_bass_kernel_spmd

    pred = np.ascontiguousarray(np.asarray(pred, dtype=np.float32))
    positive = np.ascontiguousarray(np.asarray(positive, dtype=np.float32))

    zpT, zqT, s = _host_prep(pred, positive)

    nc = _get_nc()
    in_maps = []
    for c in range(NCORES):
        k = c * M_LOCAL
        in_maps.append({
            "zpt": np.concatenate([zpT[:, k:], zpT[:, :k]], axis=1),
            "zqt": np.concatenate([zqT[:, k:], zqT[:, :k]], axis=1),
        })
    res = run_bass_kernel_spmd(nc, in_maps, core_ids=list(range(NCORES)))
    LAST_RESULTS = res

    # ---- unshard: combine per-core [128, 64] row-sum partials ----
    e2 = np.exp(np.float64(2.0))
    loss_sum = np.float64(0.0)
    for c in range(NCORES):
        o = np.asarray(res.results[c]["out"], dtype=np.float64)
        rowsum = o.reshape(P, T_OWN, 8).sum(axis=2)          # [p, m]
        neg = rowsum - e2
        # row (p, m) of core c is global row c*1024 + m*128 + p
        rows = (c * M_LOCAL
                + np.arange(T_OWN)[None, :] * P
                + np.arange(P)[:, None])
        loss_sum += np.sum(np.log(neg) - 2.0 * s[rows])
    return np.float32(loss_sum / N)



# revision 3
# speedup vs baseline: 1.0515x; 1.0515x over previous
"""SimCLR contrastive loss on 8 TRN2 NeuronCores — v3.

Same math as v2 (column-layout fp8e4 DoubleRow sims; ACT exp->bf16 and DVE
Schraudolph->fp16-codes split; PE transposed-ones chunk reduces; pp symmetry
with offsets 1..3 column sums on DVE). v3 decouples the two consumer
pipelines: ACT gets its own 2-deep stream of [128,1024] PSUM slots, DVE a
3-deep stream of [128,512] slots, so neither engine ever waits on a slot
freed by the other. Chunk reduces are emitted one unit late (deps already
satisfied -> no PE stalls propagate). Chains are per-stream PSUM columns.
"""

import numpy as np

N = 8192
D = 128
DH = 64
NCORES = 8
M = N // NCORES
P = 128
BC_A = 1024                # ACT slot columns
BC_D = 512                 # DVE slot columns
PP_JB = 40
PQ_JB = 64
NJB = PP_JB + PQ_JB        # 104
SYM_LO, SYM_HI = 8, 32
NSYM = SYM_HI - SYM_LO     # 24
OUT_COLS = 16 + NSYM       # chains(16) + colsums(24)

EPS = 1e-8
LOG2E = 1.4426950408889634
A16 = 2.0 * LOG2E * 1024.0
B16 = 15360.0 - np.log2(1.040677) * 1024.0 + np.log2(1.00645) * 1024.0

COST_ACT = 1070.0          # ns per 1024-col ACT block
COST_DVE = 760.0           # ns per 512-col DVE half-block
COST_CSUM = 436.0          # DVE colsum per sym block

_CACHE = {}


import os
BIAS = float(os.environ.get("SPLIT_BIAS", "2000"))


def _split():
    """Assign j-blocks: sym blocks forced to ACT, rest greedy by makespan.
    Returns (act_blocks, dve_blocks) in ascending order (pp first -> DMA
    overlap)."""
    act = list(range(SYM_LO, SYM_HI))
    t_act = len(act) * COST_ACT + BIAS
    t_dve = len(act) * COST_CSUM   # colsums land on DVE
    dve = []
    for n in list(range(0, SYM_LO)) + list(range(SYM_HI, NJB)):
        if t_act + COST_ACT <= t_dve + 2 * COST_DVE:
            act.append(n)
            t_act += COST_ACT
        else:
            dve.append(n)
            t_dve += 2 * COST_DVE
    return sorted(act), sorted(dve)


def _build_nc():
    import concourse.mybir as mybir
    from concourse import bacc
    from concourse.tile import TileContext
    from contextlib import ExitStack

    f32 = mybir.dt.float32
    bf16 = mybir.dt.bfloat16
    fp16 = mybir.dt.float16
    i16 = mybir.dt.int16
    fp8e4 = mybir.dt.float8e4
    AF = mybir.ActivationFunctionType
    A = mybir.AluOpType
    DR = mybir.MatmulPerfMode.DoubleRow

    act_blocks, dve_blocks = _split()
    na, nd = len(act_blocks), 2 * len(dve_blocks)   # units: blocks / halves
    dve_halves = [(n, h) for n in dve_blocks for h in (0, 1)]

    nc = bacc.Bacc()
    zpp_d = nc.dram_tensor("zpp", [DH, 2, PP_JB * P], fp8e4, kind="ExternalInput")
    zq_d = nc.dram_tensor("zq", [DH, 2, N], fp8e4, kind="ExternalInput")
    out_d = nc.dram_tensor("out", [P, OUT_COLS], f32, kind="ExternalOutput")

    with TileContext(nc) as tc:
        with ExitStack() as ctx:
            sbuf = ctx.enter_context(tc.tile_pool(name="sbuf", bufs=1))
            zppt = sbuf.tile([DH, 2, PP_JB * P], fp8e4)
            zqt = sbuf.tile([DH, 2, N], fp8e4)
            ones_bf = sbuf.tile([P, 1], bf16)
            ones_fp = sbuf.tile([P, 1], fp16)
            junk_bf = sbuf.tile([P, BC_A], bf16)
            csum = sbuf.tile([P, NSYM], f32)
            chains_sb = sbuf.tile([P, 16], f32)

            nc.vector.memset(ones_bf[:, :], 1.0)
            nc.vector.memset(ones_fp[:, :], 1.0)

            nc.sync.dma_start(out=zppt[:, :, :512], in_=zpp_d[:, :, :512])
            nc.sync.dma_start(out=zppt[:, :, 512:BC_A], in_=zpp_d[:, :, 512:BC_A])
            nc.sync.dma_start(out=zppt[:, :, BC_A:], in_=zpp_d[:, :, BC_A:])
            for g in range(4):
                cs = slice(g * (N // 4), (g + 1) * (N // 4))
                nc.sync.dma_start(out=zqt[:, :, cs], in_=zq_d[:, :, cs])

            ps_a = ctx.enter_context(tc.tile_pool(name="ps_a", bufs=2,
                                                  space="PSUM"))
            ps_d = ctx.enter_context(tc.tile_pool(name="ps_d", bufs=3,
                                                  space="PSUM"))
            ps_c = ctx.enter_context(tc.tile_pool(name="ps_c", bufs=1,
                                                  space="PSUM"))
            chains = ps_c.tile([P, 16], f32)
            act_st = ctx.enter_context(tc.tile_pool(name="act_st", bufs=3))
            dve_st = ctx.enter_context(tc.tile_pool(name="dve_st", bufs=3))

            def lhsT(n):
                if n < PP_JB:
                    return zppt[:, :, n * P:(n + 1) * P]
                j = n - PP_JB
                return zqt[:, :, j * P:(j + 1) * P]

            def sims_a(ii, pt):
                n = act_blocks[ii]
                for s in range(2):
                    nc.tensor.matmul(
                        pt[:, s * 512:(s + 1) * 512], lhsT=lhsT(n),
                        rhs=zppt[:, :, s * 512:(s + 1) * 512],
                        start=True, stop=True, perf_mode=DR)

            def sims_d(ii, pt):
                n, h = dve_halves[ii]
                nc.tensor.matmul(
                    pt[:, :], lhsT=lhsT(n),
                    rhs=zppt[:, :, h * 512:(h + 1) * 512],
                    start=True, stop=True, perf_mode=DR)

            DEPTH_A, DEPTH_D = 2, 3
            slots_a, slots_d = [], []
            for ii in range(min(DEPTH_A, na)):
                pt = ps_a.tile([P, BC_A], f32, tag="sa")
                sims_a(ii, pt)
                slots_a.append(pt)
            for ii in range(min(DEPTH_D, nd)):
                pt = ps_d.tile([P, BC_D], f32, tag="sd")
                sims_d(ii, pt)
                slots_d.append(pt)

            # deferred PE/colsum work: (kind, payload)
            pending = []

            def flush_pending():
                for kind, pl in pending:
                    if kind == "ca":       # ACT chunk reduces + maybe colsum
                        ia, st = pl
                        n = act_blocks[ia]
                        for k in range(8):
                            nc.tensor.matmul(
                                chains[:, k:k + 1],
                                lhsT=st[:, k * P:(k + 1) * P],
                                rhs=ones_bf[:, :],
                                start=(ia == 0), stop=(ia == na - 1))
                        if SYM_LO <= n < SYM_HI:
                            nc.vector.tensor_scalar(
                                junk_bf[:, :], st[:, :], 1.0, 0.0,
                                A.mult, A.add,
                                accum_out=csum[:, n - SYM_LO:n - SYM_LO + 1])
                    else:                  # DVE chunk reduces
                        jj, st = pl
                        _, h = dve_halves[jj]
                        red = st.bitcast(fp16)
                        first = jj <= 1
                        last = jj >= nd - 2
                        for k in range(4):
                            nc.tensor.matmul(
                                chains[:, 8 + h * 4 + k:8 + h * 4 + k + 1],
                                lhsT=red[:, k * P:(k + 1) * P],
                                rhs=ones_fp[:, :],
                                start=first, stop=last)
                pending.clear()

            ia = iid = 0
            while ia < na or iid < nd:
                # ratio walk keeps both engines fed in program order
                pick_a = ia < na and (iid >= nd or ia * nd <= iid * na)
                if pick_a:
                    pt = slots_a[ia]
                    st = act_st.tile([P, BC_A], bf16, tag="a")
                    nc.scalar.activation(st[:, :], pt[:, :], AF.Exp, scale=2.0)
                    if ia + DEPTH_A < na:
                        pt2 = ps_a.tile([P, BC_A], f32, tag="sa")
                        sims_a(ia + DEPTH_A, pt2)
                        slots_a.append(pt2)
                    flush_pending()
                    pending.append(("ca", (ia, st)))
                    ia += 1
                else:
                    pt = slots_d[iid]
                    st = dve_st.tile([P, BC_D], i16, tag="d")
                    nc.vector.tensor_scalar(st[:, :], pt[:, :], float(A16),
                                            float(B16), A.mult, A.add)
                    if iid + DEPTH_D < nd:
                        pt2 = ps_d.tile([P, BC_D], f32, tag="sd")
                        sims_d(iid + DEPTH_D, pt2)
                        slots_d.append(pt2)
                    flush_pending()
                    pending.append(("cd", (iid, st)))
                    iid += 1
            flush_pending()

            nc.scalar.activation(chains_sb[:, :], chains[:, :], AF.Copy)
            nc.sync.dma_start(out=out_d[:, 0:16], in_=chains_sb[:, :])
            nc.sync.dma_start(out=out_d[:, 16:OUT_COLS], in_=csum[:, :])

    nc.finalize()
    return nc


def _get_nc():
    if "nc" not in _CACHE:
        _CACHE["nc"] = _build_nc()
    return _CACHE["nc"]


def _host_prep(pred, positive):
    import ml_dtypes

    def nrm(x):
        n = np.sqrt(np.sum(x * x, axis=1, keepdims=True))
        return x / np.maximum(n, np.float32(EPS))

    zp = nrm(pred)
    zq = nrm(positive)
    s = np.sum(zp.astype(np.float64) * zq.astype(np.float64), axis=1)

    def il(mT):
        return np.ascontiguousarray(
            np.stack([mT[:DH], mT[DH:]], axis=1)).astype(
                ml_dtypes.float8_e4m3fn)

    zq_il = il(zq.T)
    in_maps = []
    for c in range(NCORES):
        k = c * M
        zroll = np.concatenate([zp[k:], zp[:k]], axis=0)[:PP_JB * P]
        in_maps.append({"zpp": il(zroll.T), "zq": zq_il})
    return in_maps, s


LAST_RESULTS = None


def kernel(pred: np.ndarray, positive: np.ndarray) -> np.ndarray:
    global LAST_RESULTS
    import sys
    if "/opt/trn_rl_repo" not in sys.path:
        sys.path.insert(0, "/opt/trn_rl_repo")
    from concourse.bass_utils import run_bass_kernel_spmd

    pred = np.ascontiguousarray(np.asarray(pred, dtype=np.float32))
    positive = np.ascontiguousarray(np.asarray(positive, dtype=np.float32))

    in_maps, s = _host_prep(pred, positive)
    nc = _get_nc()
    res = run_bass_kernel_spmd(nc, in_maps, core_ids=list(range(NCORES)))
    LAST_RESULTS = res

    neg = np.zeros(N, dtype=np.float64)
    pidx = np.arange(P)
    for c in range(NCORES):
        o = np.asarray(res.results[c]["out"], dtype=np.float64)
        base = c * M
        for k in range(8):
            neg[base + k * P + pidx] += o[:, k] + o[:, 8 + k]
        for t in range(NSYM):
            j = (base + (SYM_LO + t) * P + pidx) % N
            neg[j] += o[:, 16 + t]
    neg -= np.exp(2.0)
    loss = np.mean(np.log(neg) - 2.0 * s)
    return np.float32(loss)


# revision 4
# speedup vs baseline: 1.0642x; 1.0121x over previous
"""SimCLR contrastive loss on 8 TRN2 NeuronCores — v3.

Same math as v2 (column-layout fp8e4 DoubleRow sims; ACT exp->bf16 and DVE
Schraudolph->fp16-codes split; PE transposed-ones chunk reduces; pp symmetry
with offsets 1..3 column sums on DVE). v3 decouples the two consumer
pipelines: ACT gets its own 2-deep stream of [128,1024] PSUM slots, DVE a
3-deep stream of [128,512] slots, so neither engine ever waits on a slot
freed by the other. Chunk reduces are emitted one unit late (deps already
satisfied -> no PE stalls propagate). Chains are per-stream PSUM columns.
"""

import numpy as np

N = 8192
D = 128
DH = 64
NCORES = 8
M = N // NCORES
P = 128
BC_A = 1024                # ACT slot columns
BC_D = 512                 # DVE slot columns
PP_JB = 40
PQ_JB = 64
NJB = PP_JB + PQ_JB        # 104
SYM_LO, SYM_HI = 8, 32
NSYM = SYM_HI - SYM_LO     # 24
OUT_COLS = 16 + NSYM       # chains(16) + colsums(24)

EPS = 1e-8
LOG2E = 1.4426950408889634
A16 = 2.0 * LOG2E * 1024.0
B16 = 15360.0 - np.log2(1.040677) * 1024.0 + np.log2(1.00645) * 1024.0

COST_ACT = 1070.0          # ns per 1024-col ACT block
COST_DVE = 760.0           # ns per 512-col DVE half-block
COST_CSUM = 436.0          # DVE colsum per sym block

_CACHE = {}


import os
BIAS = float(os.environ.get("SPLIT_BIAS", "14000"))


def _split():
    """Assign j-blocks: sym blocks forced to ACT, rest greedy by makespan.
    Returns (act_blocks, dve_blocks) in ascending order (pp first -> DMA
    overlap)."""
    act = list(range(SYM_LO, SYM_HI))
    t_act = len(act) * COST_ACT + BIAS
    t_dve = len(act) * COST_CSUM   # colsums land on DVE
    dve = []
    for n in list(range(0, SYM_LO)) + list(range(SYM_HI, NJB)):
        if t_act + COST_ACT <= t_dve + 2 * COST_DVE:
            act.append(n)
            t_act += COST_ACT
        else:
            dve.append(n)
            t_dve += 2 * COST_DVE
    return sorted(act), sorted(dve)


def _build_nc():
    import concourse.mybir as mybir
    from concourse import bacc
    from concourse.tile import TileContext
    from contextlib import ExitStack

    f32 = mybir.dt.float32
    bf16 = mybir.dt.bfloat16
    fp16 = mybir.dt.float16
    i16 = mybir.dt.int16
    fp8e4 = mybir.dt.float8e4
    AF = mybir.ActivationFunctionType
    A = mybir.AluOpType
    DR = mybir.MatmulPerfMode.DoubleRow

    act_blocks, dve_blocks = _split()
    na, nd = len(act_blocks), 2 * len(dve_blocks)   # units: blocks / halves
    dve_halves = [(n, h) for n in dve_blocks for h in (0, 1)]

    nc = bacc.Bacc()
    zpp_d = nc.dram_tensor("zpp", [DH, 2, PP_JB * P], fp8e4, kind="ExternalInput")
    zq_d = nc.dram_tensor("zq", [DH, 2, N], fp8e4, kind="ExternalInput")
    out_d = nc.dram_tensor("out", [P, OUT_COLS], f32, kind="ExternalOutput")

    with TileContext(nc) as tc:
        with ExitStack() as ctx:
            sbuf = ctx.enter_context(tc.tile_pool(name="sbuf", bufs=1))
            zppt = sbuf.tile([DH, 2, PP_JB * P], fp8e4)
            zqt = sbuf.tile([DH, 2, N], fp8e4)
            ones_bf = sbuf.tile([P, 1], bf16)
            ones_fp = sbuf.tile([P, 1], fp16)
            junk_bf = sbuf.tile([P, BC_A], bf16)
            csum = sbuf.tile([P, NSYM], f32)
            chains_sb = sbuf.tile([P, 16], f32)

            nc.vector.memset(ones_bf[:, :], 1.0)
            nc.vector.memset(ones_fp[:, :], 1.0)

            nc.sync.dma_start(out=zppt[:, :, :512], in_=zpp_d[:, :, :512])
            nc.sync.dma_start(out=zppt[:, :, 512:BC_A], in_=zpp_d[:, :, 512:BC_A])
            nc.sync.dma_start(out=zppt[:, :, BC_A:], in_=zpp_d[:, :, BC_A:])
            for g in range(4):
                cs = slice(g * (N // 4), (g + 1) * (N // 4))
                nc.sync.dma_start(out=zqt[:, :, cs], in_=zq_d[:, :, cs])

            ps_a = ctx.enter_context(tc.tile_pool(name="ps_a", bufs=2,
                                                  space="PSUM"))
            ps_d = ctx.enter_context(tc.tile_pool(name="ps_d", bufs=3,
                                                  space="PSUM"))
            ps_c = ctx.enter_context(tc.tile_pool(name="ps_c", bufs=1,
                                                  space="PSUM"))
            chains = ps_c.tile([P, 16], f32)
            act_st = ctx.enter_context(tc.tile_pool(name="act_st", bufs=3))
            dve_st = ctx.enter_context(tc.tile_pool(name="dve_st", bufs=3))

            def lhsT(n):
                if n < PP_JB:
                    return zppt[:, :, n * P:(n + 1) * P]
                j = n - PP_JB
                return zqt[:, :, j * P:(j + 1) * P]

            def sims_a(ii, pt):
                n = act_blocks[ii]
                for s in range(2):
                    nc.tensor.matmul(
                        pt[:, s * 512:(s + 1) * 512], lhsT=lhsT(n),
                        rhs=zppt[:, :, s * 512:(s + 1) * 512],
                        start=True, stop=True, perf_mode=DR)

            def sims_d(ii, pt):
                n, h = dve_halves[ii]
                nc.tensor.matmul(
                    pt[:, :], lhsT=lhsT(n),
                    rhs=zppt[:, :, h * 512:(h + 1) * 512],
                    start=True, stop=True, perf_mode=DR)

            DEPTH_A, DEPTH_D = 2, 3
            slots_a, slots_d = [], []
            for ii in range(min(DEPTH_A, na)):
                pt = ps_a.tile([P, BC_A], f32, tag="sa")
                sims_a(ii, pt)
                slots_a.append(pt)
            for ii in range(min(DEPTH_D, nd)):
                pt = ps_d.tile([P, BC_D], f32, tag="sd")
                sims_d(ii, pt)
                slots_d.append(pt)

            # deferred PE/colsum work: (kind, payload)
            pending = []

            def flush_pending():
                for kind, pl in pending:
                    if kind == "ca":       # ACT chunk reduces + maybe colsum
                        ia, st = pl
                        n = act_blocks[ia]
                        for k in range(8):
                            nc.tensor.matmul(
                                chains[:, k:k + 1],
                                lhsT=st[:, k * P:(k + 1) * P],
                                rhs=ones_bf[:, :],
                                start=(ia == 0), stop=(ia == na - 1))
                        if SYM_LO <= n < SYM_HI:
                            nc.vector.tensor_scalar(
                                junk_bf[:, :], st[:, :], 1.0, 0.0,
                                A.mult, A.add,
                                accum_out=csum[:, n - SYM_LO:n - SYM_LO + 1])
                    else:                  # DVE chunk reduces
                        jj, st = pl
                        _, h = dve_halves[jj]
                        red = st.bitcast(fp16)
                        first = jj <= 1
                        last = jj >= nd - 2
                        for k in range(4):
                            nc.tensor.matmul(
                                chains[:, 8 + h * 4 + k:8 + h * 4 + k + 1],
                                lhsT=red[:, k * P:(k + 1) * P],
                                rhs=ones_fp[:, :],
                                start=first, stop=last)
                pending.clear()

            ia = iid = 0
            while ia < na or iid < nd:
                # ratio walk keeps both engines fed in program order
                pick_a = ia < na and (iid >= nd or ia * nd <= iid * na)
                if pick_a:
                    pt = slots_a[ia]
                    st = act_st.tile([P, BC_A], bf16, tag="a")
                    nc.scalar.activation(st[:, :], pt[:, :], AF.Exp, scale=2.0)
                    if ia + DEPTH_A < na:
                        pt2 = ps_a.tile([P, BC_A], f32, tag="sa")
                        sims_a(ia + DEPTH_A, pt2)
                        slots_a.append(pt2)
                    flush_pending()
                    pending.append(("ca", (ia, st)))
                    ia += 1
                else:
                    pt = slots_d[iid]
                    st = dve_st.tile([P, BC_D], i16, tag="d")
                    nc.vector.tensor_scalar(st[:, :], pt[:, :], float(A16),
                                            float(B16), A.mult, A.add)
                    if iid + DEPTH_D < nd:
                        pt2 = ps_d.tile([P, BC_D], f32, tag="sd")
                        sims_d(iid + DEPTH_D, pt2)
                        slots_d.append(pt2)
                    flush_pending()
                    pending.append(("cd", (iid, st)))
                    iid += 1
            flush_pending()

            nc.scalar.activation(chains_sb[:, :], chains[:, :], AF.Copy)
            nc.sync.dma_start(out=out_d[:, 0:16], in_=chains_sb[:, :])
            nc.sync.dma_start(out=out_d[:, 16:OUT_COLS], in_=csum[:, :])

    nc.finalize()
    return nc


def _get_nc():
    if "nc" not in _CACHE:
        _CACHE["nc"] = _build_nc()
    return _CACHE["nc"]


def _host_prep(pred, positive):
    import ml_dtypes

    def nrm(x):
        n = np.sqrt(np.sum(x * x, axis=1, keepdims=True))
        return x / np.maximum(n, np.float32(EPS))

    zp = nrm(pred)
    zq = nrm(positive)
    s = np.sum(zp.astype(np.float64) * zq.astype(np.float64), axis=1)

    def il(mT):
        return np.ascontiguousarray(
            np.stack([mT[:DH], mT[DH:]], axis=1)).astype(
                ml_dtypes.float8_e4m3fn)

    zq_il = il(zq.T)
    in_maps = []
    for c in range(NCORES):
        k = c * M
        zroll = np.concatenate([zp[k:], zp[:k]], axis=0)[:PP_JB * P]
        in_maps.append({"zpp": il(zroll.T), "zq": zq_il})
    return in_maps, s


LAST_RESULTS = None


def kernel(pred: np.ndarray, positive: np.ndarray) -> np.ndarray:
    global LAST_RESULTS
    import sys
    if "/opt/trn_rl_repo" not in sys.path:
        sys.path.insert(0, "/opt/trn_rl_repo")
    from concourse.bass_utils import run_bass_kernel_spmd

    pred = np.ascontiguousarray(np.asarray(pred, dtype=np.float32))
    positive = np.ascontiguousarray(np.asarray(positive, dtype=np.float32))

    in_maps, s = _host_prep(pred, positive)
    nc = _get_nc()
    res = run_bass_kernel_spmd(nc, in_maps, core_ids=list(range(NCORES)))
    LAST_RESULTS = res

    neg = np.zeros(N, dtype=np.float64)
    pidx = np.arange(P)
    for c in range(NCORES):
        o = np.asarray(res.results[c]["out"], dtype=np.float64)
        base = c * M
        for k in range(8):
            neg[base + k * P + pidx] += o[:, k] + o[:, 8 + k]
        for t in range(NSYM):
            j = (base + (SYM_LO + t) * P + pidx) % N
            neg[j] += o[:, 16 + t]
    neg -= np.exp(2.0)
    loss = np.mean(np.log(neg) - 2.0 * s)
    return np.float32(loss)
